# revision 2
# baseline (speedup 1.0000x reference)
"""Distributed MHA kernel for 8 Trainium2 NeuronCores (v2).

Sharding: core i handles batch b = i//2, head-group g = i%2 (8 of 16 heads).
Data parallel on B, tensor parallel on H: column-parallel QKV, row-parallel
output projection with the partial sums reduced on the host during gather.

Math (per core, heads h in its group, E=1024, H=16, d=64, N=1024):
  QT[hd, n] = sum_e Wq[hd, e] x[n, e] + bq[hd]        (transposed layout)
  KT[hd, n] = likewise
  V[n, hd]  = sum_e x[n, e] Wv[hd, e] / 32             (bv folded on host,
                                                        1/32 att scale folded)
  energyT_h[k, q] = sum_d KT_h[d, k] QT_h[d, q]
  expT_h = exp(energyT_h)          (no max-subtract; |energy| < ~50 is safe)
  po_h[q, 0:64] = sum_k expT_h[k, q] V_h[k, d]         (full-K matmul,
  po_h[q, 64]   = sum_k expT_h[k, q]                    ones column in V)
  norm_h[q, d]  = po_h[q, 0:64] * (1/po_h[q, 64])      (per-partition scalar)
  packT[he, q]  = norm[q, he].T                        (DMA crossbar transpose)
  y_part[q, e]  = sum_{he} packT[he, q] Wo[e, he]
Host: out[b] = y_part[2b] + y_part[2b+1] + (bo + Wo @ bv / 32)

Schedule: the Act engine (exp, ~64us) paces stage B; QKV/V/output-projection
matmuls are interleaved into the energy stream so the PE never idles long.
Energy matmuls for the two heads of a pair sit on PE row-groups (0,0)/(64,0)
and run concurrently on hardware.
"""

import numpy as np
import ml_dtypes

import concourse.bass as bass
import concourse.tile as tile
from concourse import mybir
from concourse.bass_utils import run_bass_kernel_spmd

E = 1024
N = 1024
B = 4
NC = 8
EH = 512          # head dims per core (8 heads x 64)
D = 64
BF16 = mybir.dt.bfloat16
F32 = mybir.dt.float32
F32R = mybir.dt.float32r
AX = mybir.AluOpType
ACT = mybir.ActivationFunctionType


def split_drain_waits(nc):
    """Walrus in this toolchain rejects instructions carrying more than one
    sem wait; move extra waits onto injected same-engine NOPs placed right
    before the instruction (same engine queue = program order preserved)."""
    def take_nop(engine):
        nop = nc.engines[engine].nop(nofuse=True).ins
        for bname, bw in nc.bb_map.items():
            lst = bw.bb.instructions
            if lst and lst[-1].name == nop.name:
                bw.bb.instructions = lst[:-1]
                break
        return nop

    for name, w in nc.bb_map.items():
        bb = w.bb
        new_insts = []
        changed = False
        for ins in bb.instructions:
            si = ins.sync_info
            if si is not None and si.on_wait and len(si.on_wait) > 1:
                waits = list(si.on_wait)
                for wt in waits[:-1]:
                    nop = take_nop(ins.engine)
                    nop.sync_info = mybir.SyncInfo(on_wait=[wt], on_update=[])
                    new_insts.append(nop)
                si.on_wait = waits[-1:]
                ins.sync_info = si
                changed = True
            new_insts.append(ins)
        if changed:
            bb.instructions = new_insts


def _emit(nc: bass.Bass, tc: tile.TileContext, ctx):
    xT = nc.declare_dram_parameter("xT", [E, N], F32R, isOutput=False)
    wqT = nc.declare_dram_parameter("wqT", [E, EH], F32R, isOutput=False)
    wkT = nc.declare_dram_parameter("wkT", [E, EH], F32R, isOutput=False)
    wvT = nc.declare_dram_parameter("wvT", [E, EH], F32R, isOutput=False)
    woT = nc.declare_dram_parameter("woT", [EH, E], BF16, isOutput=False)
    bqd = nc.declare_dram_parameter("bq", [4, 128, 1], F32, isOutput=False)
    bkd = nc.declare_dram_parameter("bk", [4, 128, 1], F32, isOutput=False)
    y = nc.declare_dram_parameter("y", [N, E], BF16, isOutput=True)

    persist = ctx.enter_context(tc.tile_pool(name="persist", bufs=1))
    expool = ctx.enter_context(tc.tile_pool(name="expool", bufs=1))
    work = ctx.enter_context(tc.tile_pool(name="work", bufs=1))
    ps = ctx.enter_context(tc.tile_pool(name="ps", bufs=1, space="PSUM"))

    # ---- input DMA (3 queues: SP, Act-HWDGE, gpsimd SWDGE) ----
    # Priority: wq/wk m0 column slices + x (unblock first energy), then the
    # rest.  xt split across the two HW queues.
    wqm = [[None] * 4 for _ in range(8)]
    wkm = [[None] * 4 for _ in range(8)]
    for e in range(8):
        t = persist.tile([128, 128], F32R, tag=f"wqm{e}_0", name=f"wqm{e}_0")
        nc.sync.dma_start(out=t, in_=wqT[e * 128:(e + 1) * 128, 0:128])
        wqm[e][0] = t
        t = persist.tile([128, 128], F32R, tag=f"wkm{e}_0", name=f"wkm{e}_0")
        nc.scalar.dma_start(out=t, in_=wkT[e * 128:(e + 1) * 128, 0:128])
        wkm[e][0] = t
    xt = []
    for e in range(8):
        t = persist.tile([128, N], F32R, tag=f"xt{e}", name=f"xt{e}")
        eng = nc.sync if e % 2 == 0 else nc.scalar
        eng.dma_start(out=t, in_=xT[e * 128:(e + 1) * 128, :])
        xt.append(t)
    for m in range(1, 4):
        for e in range(8):
            t = persist.tile([128, 128], F32R, tag=f"wqm{e}_{m}", name=f"wqm{e}_{m}")
            nc.sync.dma_start(out=t, in_=wqT[e * 128:(e + 1) * 128, m * 128:(m + 1) * 128])
            wqm[e][m] = t
            t = persist.tile([128, 128], F32R, tag=f"wkm{e}_{m}", name=f"wkm{e}_{m}")
            nc.scalar.dma_start(out=t, in_=wkT[e * 128:(e + 1) * 128, m * 128:(m + 1) * 128])
            wkm[e][m] = t
    wo = []
    for p in range(4):
        t = persist.tile([128, E], BF16, tag=f"wo{p}", name=f"wo{p}")
        nc.sync.dma_start(out=t, in_=woT[p * 128:(p + 1) * 128, :])
        wo.append(t)
    wv = []
    for e in range(8):
        t = persist.tile([128, EH], F32R, tag=f"wv{e}", name=f"wv{e}")
        nc.gpsimd.dma_start(out=t, in_=wvT[e * 128:(e + 1) * 128, :])
        wv.append(t)
    bq_sb, bk_sb = [], []
    for m in range(4):
        t = persist.tile([128, 1], F32, tag=f"bq{m}", name=f"bq{m}")
        nc.gpsimd.dma_start(out=t, in_=bqd[m])
        bq_sb.append(t)
        t = persist.tile([128, 1], F32, tag=f"bk{m}", name=f"bk{m}")
        nc.gpsimd.dma_start(out=t, in_=bkd[m])
        bk_sb.append(t)

    # Warm the Exp activation table during the DMA head.
    warm = persist.tile([1, 8], F32, tag="warm", name="warm")
    nc.vector.memset(warm, 0.0)
    nc.scalar.activation(out=warm, in_=warm, func=ACT.Exp)

    # persistent compute tiles
    qt = [persist.tile([128, N], F32R, tag=f"qt{m}", name=f"qt{m}") for m in range(4)]
    kt = [persist.tile([128, N], F32R, tag=f"kt{m}", name=f"kt{m}") for m in range(4)]
    vt = [persist.tile([128, 8, 65], BF16, tag=f"v{n}", name=f"v{n}") for n in range(8)]
    packT = [[persist.tile([128, 128], BF16, tag=f"pk{p}_{q}", name=f"pk{p}_{q}")
              for q in range(8)] for p in range(4)]

    # ---- PE work chunks (fillers interleaved into the act-paced B stream) --

    def emit_qk(m):
        """QK projections for pair m as 4 lazily-allocating PE chunks.

        PSUM tiles MUST be allocated inside the chunk bodies (at emission
        position) so tag-slot rotation order matches instruction order."""
        cell = {}

        def mm(dst, wsel, half):
            for e in range(8):
                w = wqm[e][m] if wsel == 0 else wkm[e][m]
                nc.tensor.matmul(
                    out=dst[:, half * 512:(half + 1) * 512],
                    lhsT=w, rhs=xt[e][:, half * 512:(half + 1) * 512],
                    start=(e == 0), stop=(e == 7))

        def bias(dst, src, b_sb):
            for half in range(2):
                nc.vector.tensor_scalar_add(
                    dst[:, half * 512:(half + 1) * 512],
                    src[:, half * 512:(half + 1) * 512], b_sb)

        def c0():
            cell["psq"] = ps.tile([128, 1024], F32, tag="en", bufs=2,
                                  name=f"psq{m}")
            mm(cell["psq"], 0, 0)

        def c1():
            mm(cell["psq"], 0, 1)

        def c2():
            cell["psk"] = ps.tile([128, 1024], F32, tag="en", bufs=2,
                                  name=f"psk{m}")
            mm(cell["psk"], 1, 0)
            bias(qt[m], cell["psq"], bq_sb[m])

        def c3():
            mm(cell["psk"], 1, 1)
            bias(kt[m], cell["psk"], bk_sb[m])

        return [c0, c1, c2, c3]

    def emit_v(n):
        def go():
            psv = ps.tile([128, 512], F32, tag=f"cs{n % 2}", bufs=1, name=f"psv{n}")
            for e in range(8):
                nc.tensor.matmul(
                    out=psv, lhsT=xt[e][:, n * 128:(n + 1) * 128], rhs=wv[e],
                    start=(e == 0), stop=(e == 7))
            nc.vector.memset(vt[n][:, :, 64:65], 1.0)
            nc.vector.tensor_copy(
                vt[n][:, :, 0:64], psv.rearrange("p (h d) -> p h d", h=8))
        return go

    ys_ctr = [0]

    def emit_c(qt_i, es, copy_eng):
        def go():
            psy = ps.tile([128, 512], F32, tag=f"cs{es}", bufs=1, name=f"psy{qt_i}_{es}")
            for p in range(4):
                nc.tensor.matmul(
                    out=psy, lhsT=packT[p][qt_i],
                    rhs=wo[p][:, es * 512:(es + 1) * 512],
                    start=(p == 0), stop=(p == 3))
            ys = work.tile([128, 512], BF16, tag="ys", bufs=4,
                           name=f"ys{ys_ctr[0]}")
            ys_ctr[0] += 1
            if copy_eng == "act":
                nc.scalar.copy(ys, psy)
            else:
                nc.vector.tensor_copy(ys, psy)
            nc.sync.dma_start(
                out=y[qt_i * 128:(qt_i + 1) * 128, es * 512:(es + 1) * 512],
                in_=ys)
        return go

    # ---- stage B machinery ----
    def emit_en_exp(p, qs, k):
        """Energy for both heads of pair p (concurrent PE row groups) + exp."""
        en = ps.tile([128, 1024], F32, tag="en", bufs=2, name=f"en{p}_{qs}_{k}")
        for ab in range(2):
            nc.tensor.matmul(
                out=en[:, ab * 512:(ab + 1) * 512],
                lhsT=kt[p][ab * 64:(ab + 1) * 64, k * 128:(k + 1) * 128],
                rhs=qt[p][ab * 64:(ab + 1) * 64, qs * 512:(qs + 1) * 512],
                start=True, stop=True)
        ex = expool.tile([128, 1024], BF16, tag="ex", bufs=12,
                         name=f"ex{p}_{qs}_{k}")
        nc.scalar.activation(out=ex, in_=en, func=ACT.Exp)
        return ex

    def emit_attv(p, qs, qq, exs, po):
        """One qq-slice of att@V for both heads: full k-accumulation so each
        PSUM bank has a single pending accumulation group at a time."""
        for ab in range(2):
            for k in range(8):
                nc.tensor.matmul(
                    out=po[ab][:, qq, :],
                    lhsT=exs[k][:, ab * 512 + qq * 128:ab * 512 + (qq + 1) * 128],
                    rhs=vt[k][:, 2 * p + ab, :],
                    start=(k == 0), stop=(k == 7))

    def emit_norm(p, qs, po):
        """recip + per-partition scale, pack two heads, DMA-transpose."""
        s2 = work.tile([128, 2, 4], F32, tag="s2", bufs=2, name=f"s2_{p}_{qs}")
        for ab in range(2):
            nc.vector.reciprocal(out=s2[:, ab, :], in_=po[ab][:, :, 64])
        for qq in range(4):
            nt = work.tile([128, 128], BF16, tag="norm", bufs=6,
                           name=f"nt{p}_{qs}_{qq}")
            for ab in range(2):
                nc.vector.tensor_scalar_mul(
                    nt[:, ab * 64:(ab + 1) * 64],
                    po[ab][:, qq, 0:64], s2[:, ab, qq:qq + 1])
            nc.sync.dma_start_transpose(packT[p][qs * 4 + qq], nt)

    # ---- the act-paced pipeline ----
    groups = [(p, qs) for qs in range(2) for p in range(4)]
    # filler chunks, in dependency-legal order; each ~0.4-1.7us of PE work
    fillers = []
    fillers += emit_qk(1)                      # during (0,0)
    fillers += [emit_v(n) for n in range(4)]   # during (1,0)
    fillers += [emit_v(n) for n in range(4, 8)]
    fillers += emit_qk(2)                      # during (2,0)
    fillers += emit_qk(3)                      # during (3,0)
    # C chunks for qs0 qtiles become legal once attV(3,0)+transposes land,
    # i.e. during groups (1,1) onwards.
    c_qs0 = [emit_c(q, es, "dve") for q in range(4) for es in range(2)]
    c_qs1 = [emit_c(q, es, "act") for q in range(4, 8) for es in range(2)]

    for ch in emit_qk(0):
        ch()

    prev = None          # (p, qs, ex_tiles, po)
    fill_i = 0
    for gi, (p, qs) in enumerate(groups):
        po = [ps.tile([128, 4, 65], F32, tag=f"po{ab}", bufs=1,
                      name=f"po{p}_{qs}_{ab}") for ab in range(2)]
        exs = []
        for k in range(8):
            ex = emit_en_exp(p, qs, k)
            exs.append(ex)
            if prev is not None and k % 2 == 1:
                pp, pqs, pexs, ppo = prev
                emit_attv(pp, pqs, (k - 1) // 2, pexs, ppo)
                if k == 7:
                    emit_norm(pp, pqs, ppo)
            # one filler chunk per k where available
            budget = 1 if prev is not None else 2
            for _ in range(budget):
                if gi >= 5 and c_qs0:
                    c_qs0.pop(0)()
                elif fill_i < len(fillers):
                    fillers[fill_i]()
                    fill_i += 1
        prev = (p, qs, exs, po)

    # tail: attV + norm of the last group, then remaining C chunks
    pp, pqs, pexs, ppo = prev
    for qq in range(4):
        emit_attv(pp, pqs, qq, pexs, ppo)
        if fill_i < len(fillers):
            fillers[fill_i]()
            fill_i += 1
    emit_norm(pp, pqs, ppo)
    while fill_i < len(fillers):
        fillers[fill_i]()
        fill_i += 1
    for ch in c_qs0:
        ch()
    for ch in c_qs1:
        ch()


def build(split=True):
    from contextlib import ExitStack
    nc = bass.Bass()
    with tile.TileContext(nc) as tc:
        with ExitStack() as ctx:
            _emit(nc, tc, ctx)
    if split:
        split_drain_waits(nc)
    return nc


def make_in_maps(x, Wq, bq, Wk, bk, Wv, bv, Wo, bo):
    bf = ml_dtypes.bfloat16
    in_maps = []
    for i in range(NC):
        b, g = i // 2, i % 2
        sl = slice(g * EH, (g + 1) * EH)
        in_maps.append({
            "xT": np.ascontiguousarray(x[b].T),
            "wqT": np.ascontiguousarray(Wq[sl, :].T),
            "wkT": np.ascontiguousarray(Wk[sl, :].T),
            "wvT": np.ascontiguousarray(Wv[sl, :].T) / 32.0,
            "woT": np.ascontiguousarray(Wo[:, sl].T).astype(bf),
            "bq": bq[sl].reshape(4, 128, 1).astype(np.float32),
            "bk": bk[sl].reshape(4, 128, 1).astype(np.float32),
        })
    return in_maps


def gather(results, Wv_b, Wo, bv, bo):
    host_bias = (bo + Wo @ bv / 32.0).astype(np.float32)
    out = np.empty((B, N, E), np.float32)
    for b in range(B):
        out[b] = (results[2 * b]["y"].astype(np.float32)
                  + results[2 * b + 1]["y"].astype(np.float32) + host_bias)
    return out


def kernel(x, Wq, bq, Wk, bk, Wv, bv, Wo, bo):
    x, Wq, bq, Wk, bk, Wv, bv, Wo, bo = [
        np.asarray(a, np.float32) for a in (x, Wq, bq, Wk, bk, Wv, bv, Wo, bo)]
    nc = build()
    in_maps = make_in_maps(x, Wq, bq, Wk, bk, Wv, bv, Wo, bo)
    res = run_bass_kernel_spmd(nc, in_maps, list(range(NC)))
    return gather(res.results, Wv, Wo, bv, bo)


if __name__ == "__main__":
    import reference
    inputs = {k: np.asarray(v) for k, v in reference.setup_inputs().items()}
    out = kernel(**inputs)
    exp = np.asarray(reference.reference(**inputs))
    rel = np.abs(out - exp).max() / np.abs(exp).max()
    print("Relative error:", rel)


# revision 3
# speedup vs baseline: 1.0764x; 1.0764x over previous
"""Distributed MHA kernel for 8 Trainium2 NeuronCores (v2).

Sharding: core i handles batch b = i//2, head-group g = i%2 (8 of 16 heads).
Data parallel on B, tensor parallel on H: column-parallel QKV, row-parallel
output projection with the partial sums reduced on the host during gather.

Math (per core, heads h in its group, E=1024, H=16, d=64, N=1024):
  QT[hd, n] = sum_e Wq[hd, e] x[n, e] + bq[hd]        (transposed layout)
  KT[hd, n] = likewise
  V[n, hd]  = sum_e x[n, e] Wv[hd, e] / 32             (bv folded on host,
                                                        1/32 att scale folded)
  energyT_h[k, q] = sum_d KT_h[d, k] QT_h[d, q]
  expT_h = exp(energyT_h)          (no max-subtract; |energy| < ~50 is safe)
  po_h[q, 0:64] = sum_k expT_h[k, q] V_h[k, d]         (full-K matmul,
  po_h[q, 64]   = sum_k expT_h[k, q]                    ones column in V)
  norm_h[q, d]  = po_h[q, 0:64] * (1/po_h[q, 64])      (per-partition scalar)
  packT[he, q]  = norm[q, he].T                        (DMA crossbar transpose)
  y_part[q, e]  = sum_{he} packT[he, q] Wo[e, he]
Host: out[b] = y_part[2b] + y_part[2b+1] + (bo + Wo @ bv / 32)

Schedule: the Act engine (exp, ~64us) paces stage B; QKV/V/output-projection
matmuls are interleaved into the energy stream so the PE never idles long.
Energy matmuls for the two heads of a pair sit on PE row-groups (0,0)/(64,0)
and run concurrently on hardware.
"""

import numpy as np
import ml_dtypes

import concourse.bass as bass
import concourse.tile as tile
from concourse import mybir
from concourse.bass_utils import run_bass_kernel_spmd

E = 1024
N = 1024
B = 4
NC = 8
EH = 512          # head dims per core (8 heads x 64)
D = 64
BF16 = mybir.dt.bfloat16
F32 = mybir.dt.float32
F32R = mybir.dt.float32r
AX = mybir.AluOpType
ACT = mybir.ActivationFunctionType


def split_drain_waits(nc):
    """Walrus in this toolchain rejects instructions carrying more than one
    sem wait; move extra waits onto injected same-engine NOPs placed right
    before the instruction (same engine queue = program order preserved)."""
    def take_nop(engine):
        nop = nc.engines[engine].nop(nofuse=True).ins
        for bname, bw in nc.bb_map.items():
            lst = bw.bb.instructions
            if lst and lst[-1].name == nop.name:
                bw.bb.instructions = lst[:-1]
                break
        return nop

    for name, w in nc.bb_map.items():
        bb = w.bb
        new_insts = []
        changed = False
        for ins in bb.instructions:
            si = ins.sync_info
            if si is not None and si.on_wait and len(si.on_wait) > 1:
                waits = list(si.on_wait)
                for wt in waits[:-1]:
                    nop = take_nop(ins.engine)
                    nop.sync_info = mybir.SyncInfo(on_wait=[wt], on_update=[])
                    new_insts.append(nop)
                si.on_wait = waits[-1:]
                ins.sync_info = si
                changed = True
            new_insts.append(ins)
        if changed:
            bb.instructions = new_insts


def _emit(nc: bass.Bass, tc: tile.TileContext, ctx):
    xT = nc.declare_dram_parameter("xT", [E, N], F32R, isOutput=False)
    wqT = nc.declare_dram_parameter("wqT", [E, EH], F32R, isOutput=False)
    wkT = nc.declare_dram_parameter("wkT", [E, EH], F32R, isOutput=False)
    wvT = nc.declare_dram_parameter("wvT", [E, EH], F32R, isOutput=False)
    woT = nc.declare_dram_parameter("woT", [EH, E], BF16, isOutput=False)
    bqd = nc.declare_dram_parameter("bq", [4, 128, 1], F32, isOutput=False)
    bkd = nc.declare_dram_parameter("bk", [4, 128, 1], F32, isOutput=False)
    y = nc.declare_dram_parameter("y", [N, E], BF16, isOutput=True)

    persist = ctx.enter_context(tc.tile_pool(name="persist", bufs=1))
    expool = ctx.enter_context(tc.tile_pool(name="expool", bufs=1))
    work = ctx.enter_context(tc.tile_pool(name="work", bufs=1))
    ps = ctx.enter_context(tc.tile_pool(name="ps", bufs=1, space="PSUM"))

    # ---- input DMA (3 queues: SP, Act-HWDGE, gpsimd SWDGE) ----
    # Priority: wq/wk m0 column slices + x (unblock first energy), then the
    # rest.  xt split 3/3/2 across the three queues.
    wq0, wk0 = [], []
    for e in range(8):
        t = persist.tile([128, 128], F32R, tag=f"wqm{e}_0", name=f"wqm{e}_0")
        nc.sync.dma_start(out=t, in_=wqT[e * 128:(e + 1) * 128, 0:128])
        wq0.append(t)
        t = persist.tile([128, 128], F32R, tag=f"wkm{e}_0", name=f"wkm{e}_0")
        nc.scalar.dma_start(out=t, in_=wkT[e * 128:(e + 1) * 128, 0:128])
        wk0.append(t)
    xt = [None] * 8
    for e, eng in ((6, nc.gpsimd), (7, nc.gpsimd), (0, nc.sync), (1, nc.scalar),
                   (2, nc.sync), (3, nc.scalar), (4, nc.sync), (5, nc.scalar)):
        t = persist.tile([128, N], F32R, tag=f"xt{e}", name=f"xt{e}")
        eng.dma_start(out=t, in_=xT[e * 128:(e + 1) * 128, :])
        xt[e] = t
    # m1-3 column blocks as single [128, 384] tiles
    wqr, wkr = [], []
    for e in range(8):
        t = persist.tile([128, 384], F32R, tag=f"wqr{e}", name=f"wqr{e}")
        nc.sync.dma_start(out=t, in_=wqT[e * 128:(e + 1) * 128, 128:512])
        wqr.append(t)
        t = persist.tile([128, 384], F32R, tag=f"wkr{e}", name=f"wkr{e}")
        nc.scalar.dma_start(out=t, in_=wkT[e * 128:(e + 1) * 128, 128:512])
        wkr.append(t)
    wqm = [[wq0[e], wqr[e][:, 0:128], wqr[e][:, 128:256], wqr[e][:, 256:384]]
           for e in range(8)]
    wkm = [[wk0[e], wkr[e][:, 0:128], wkr[e][:, 128:256], wkr[e][:, 256:384]]
           for e in range(8)]
    wo = []
    for p in range(4):
        t = persist.tile([128, E], BF16, tag=f"wo{p}", name=f"wo{p}")
        nc.sync.dma_start(out=t, in_=woT[p * 128:(p + 1) * 128, :])
        wo.append(t)
    wv = []
    for e in range(8):
        t = persist.tile([128, EH], F32R, tag=f"wv{e}", name=f"wv{e}")
        nc.gpsimd.dma_start(out=t, in_=wvT[e * 128:(e + 1) * 128, :])
        wv.append(t)
    bq_sb, bk_sb = [], []
    for m in range(4):
        t = persist.tile([128, 1], F32, tag=f"bq{m}", name=f"bq{m}")
        nc.gpsimd.dma_start(out=t, in_=bqd[m])
        bq_sb.append(t)
        t = persist.tile([128, 1], F32, tag=f"bk{m}", name=f"bk{m}")
        nc.gpsimd.dma_start(out=t, in_=bkd[m])
        bk_sb.append(t)

    # Warm the Exp activation table during the DMA head.
    warm = persist.tile([1, 8], F32, tag="warm", name="warm")
    nc.vector.memset(warm, 0.0)
    nc.scalar.activation(out=warm, in_=warm, func=ACT.Exp)

    # persistent compute tiles
    qt = [persist.tile([128, N], F32R, tag=f"qt{m}", name=f"qt{m}") for m in range(4)]
    kt = [persist.tile([128, N], F32R, tag=f"kt{m}", name=f"kt{m}") for m in range(4)]
    vt = [persist.tile([128, 8, 65], BF16, tag=f"v{n}", name=f"v{n}") for n in range(8)]
    packT = [[persist.tile([128, 128], BF16, tag=f"pk{p}_{q}", name=f"pk{p}_{q}")
              for q in range(8)] for p in range(4)]

    # ---- PE work chunks (fillers interleaved into the act-paced B stream) --

    def emit_qk(m):
        """QK projections for pair m as 4 lazily-allocating PE chunks.

        PSUM tiles MUST be allocated inside the chunk bodies (at emission
        position) so tag-slot rotation order matches instruction order."""
        cell = {}

        def mm(dst, wsel, half):
            for e in range(8):
                w = wqm[e][m] if wsel == 0 else wkm[e][m]
                nc.tensor.matmul(
                    out=dst[:, half * 512:(half + 1) * 512],
                    lhsT=w, rhs=xt[e][:, half * 512:(half + 1) * 512],
                    start=(e == 0), stop=(e == 7))

        def bias(dst, src, b_sb):
            for half in range(2):
                nc.vector.tensor_scalar_add(
                    dst[:, half * 512:(half + 1) * 512],
                    src[:, half * 512:(half + 1) * 512], b_sb)

        def c0():
            cell["psq"] = ps.tile([128, 1024], F32, tag="en", bufs=2,
                                  name=f"psq{m}")
            mm(cell["psq"], 0, 0)

        def c1():
            mm(cell["psq"], 0, 1)

        def c2():
            cell["psk"] = ps.tile([128, 1024], F32, tag="en", bufs=2,
                                  name=f"psk{m}")
            mm(cell["psk"], 1, 0)
            bias(qt[m], cell["psq"], bq_sb[m])

        def c3():
            mm(cell["psk"], 1, 1)
            bias(kt[m], cell["psk"], bk_sb[m])

        return [c0, c1, c2, c3]

    def emit_v(n):
        def go():
            psv = ps.tile([128, 512], F32, tag=f"cs{n % 2}", bufs=1, name=f"psv{n}")
            for e in range(8):
                nc.tensor.matmul(
                    out=psv, lhsT=xt[e][:, n * 128:(n + 1) * 128], rhs=wv[e],
                    start=(e == 0), stop=(e == 7))
            nc.vector.memset(vt[n][:, :, 64:65], 1.0)
            nc.vector.tensor_copy(
                vt[n][:, :, 0:64], psv.rearrange("p (h d) -> p h d", h=8))
        return go

    ys_ctr = [0]

    def emit_c(qt_i, es, copy_eng):
        def go():
            psy = ps.tile([128, 512], F32, tag=f"cs{es}", bufs=1, name=f"psy{qt_i}_{es}")
            for p in range(4):
                nc.tensor.matmul(
                    out=psy, lhsT=packT[p][qt_i],
                    rhs=wo[p][:, es * 512:(es + 1) * 512],
                    start=(p == 0), stop=(p == 3))
            ys = work.tile([128, 512], BF16, tag="ys", bufs=4,
                           name=f"ys{ys_ctr[0]}")
            ys_ctr[0] += 1
            if copy_eng == "act":
                nc.scalar.copy(ys, psy)
            else:
                nc.vector.tensor_copy(ys, psy)
            nc.sync.dma_start(
                out=y[qt_i * 128:(qt_i + 1) * 128, es * 512:(es + 1) * 512],
                in_=ys)
        return go

    # ---- stage B machinery ----
    def emit_en_exp(p, qs, k):
        """Energy for both heads of pair p (concurrent PE row groups) + exp."""
        en = ps.tile([128, 1024], F32, tag="en", bufs=2, name=f"en{p}_{qs}_{k}")
        for ab in range(2):
            nc.tensor.matmul(
                out=en[:, ab * 512:(ab + 1) * 512],
                lhsT=kt[p][ab * 64:(ab + 1) * 64, k * 128:(k + 1) * 128],
                rhs=qt[p][ab * 64:(ab + 1) * 64, qs * 512:(qs + 1) * 512],
                start=True, stop=True)
        ex = expool.tile([128, 1024], BF16, tag="ex", bufs=12,
                         name=f"ex{p}_{qs}_{k}")
        nc.scalar.activation(out=ex, in_=en, func=ACT.Exp)
        return ex

    def emit_attv(p, qs, qq, exs, po):
        """One qq-slice of att@V for both heads: full k-accumulation so each
        PSUM bank has a single pending accumulation group at a time."""
        for ab in range(2):
            for k in range(8):
                nc.tensor.matmul(
                    out=po[ab][:, qq, :],
                    lhsT=exs[k][:, ab * 512 + qq * 128:ab * 512 + (qq + 1) * 128],
                    rhs=vt[k][:, 2 * p + ab, :],
                    start=(k == 0), stop=(k == 7))

    def emit_norm_qq(p, qs, qq, po):
        """recip + per-partition scale for one qq, pack two heads, transpose."""
        s2 = work.tile([128, 2], F32, tag="s2", bufs=4, name=f"s2_{p}_{qs}_{qq}")
        nt = work.tile([128, 128], BF16, tag="norm", bufs=6,
                       name=f"nt{p}_{qs}_{qq}")
        for ab in range(2):
            nc.vector.reciprocal(out=s2[:, ab:ab + 1], in_=po[ab][:, qq, 64:65])
            nc.vector.tensor_scalar_mul(
                nt[:, ab * 64:(ab + 1) * 64],
                po[ab][:, qq, 0:64], s2[:, ab:ab + 1])
        nc.sync.dma_start_transpose(packT[p][qs * 4 + qq], nt)

    # ---- the act-paced pipeline ----
    # qk(p) due before group index p; V due gi0 (attV(0,0) runs during gi1).
    groups = [(p, qs) for qs in range(2) for p in range(4)]
    qk1 = emit_qk(1)
    vs = [emit_v(n) for n in range(8)]
    fillers = [qk1[0], vs[0], qk1[1], vs[1], qk1[2], vs[2], qk1[3], vs[3]]
    fillers += vs[4:]
    fillers += emit_qk(2) + emit_qk(3)
    # pops per (gi, k) from the fillers queue
    fill_sched = {
        0: [2, 2, 2, 2, 1, 1, 1, 1],          # qk1 + V (12 chunks)
        1: [1, 0, 1, 0, 1, 0, 1, 0],          # qk2 (due before gi2)
        2: [1, 0, 1, 0, 1, 0, 1, 0],          # qk3 (due before gi3)
    }
    # C chunks for qs0: legal once pair-3-qs0 transposes land (during gi4);
    # spread over gi5-gi7.  qs1 in the tail.
    c_qs0 = [emit_c(q, es, "dve") for q in range(4) for es in range(2)]
    c_qs1 = [emit_c(q, es, "act") for q in range(4, 8) for es in range(2)]
    c_sched = {
        5: [0, 1, 0, 1, 0, 1, 0, 1],
        6: [0, 1, 0, 1, 0, 1, 0, 1],
    }

    for ch in emit_qk(0):
        ch()

    prev = None          # (p, qs, ex_tiles, po)
    fill_i = 0
    for gi, (p, qs) in enumerate(groups):
        po = [ps.tile([128, 4, 65], F32, tag=f"po{ab}", bufs=1,
                      name=f"po{p}_{qs}_{ab}") for ab in range(2)]
        exs = []
        for k in range(8):
            ex = emit_en_exp(p, qs, k)
            exs.append(ex)
            if prev is not None and k % 2 == 1:
                pp, pqs, pexs, ppo = prev
                qq = (k - 1) // 2
                emit_attv(pp, pqs, qq, pexs, ppo)
                if k == 7:
                    for q4 in range(4):
                        emit_norm_qq(pp, pqs, q4, ppo)
            for _ in range(fill_sched.get(gi, [0] * 8)[k]):
                if fill_i < len(fillers):
                    fillers[fill_i]()
                    fill_i += 1
            for _ in range(c_sched.get(gi, [0] * 8)[k]):
                if c_qs0:
                    c_qs0.pop(0)()
        prev = (p, qs, exs, po)

    # tail: attV + norm of the last group interleaved with the final C chunks
    pp, pqs, pexs, ppo = prev
    for qq in range(4):
        emit_attv(pp, pqs, qq, pexs, ppo)
        emit_norm_qq(pp, pqs, qq, ppo)
        if qq >= 1:
            c_qs1.pop(0)()          # C(qt4+qq-1, es0) once its packT landed
            c_qs1.pop(0)()
    while fill_i < len(fillers):
        fillers[fill_i]()
        fill_i += 1
    for ch in c_qs0:
        ch()
    for ch in c_qs1:
        ch()


def build(split=True):
    from contextlib import ExitStack
    nc = bass.Bass()
    with tile.TileContext(nc) as tc:
        with ExitStack() as ctx:
            _emit(nc, tc, ctx)
    if split:
        split_drain_waits(nc)
    return nc


def make_in_maps(x, Wq, bq, Wk, bk, Wv, bv, Wo, bo):
    bf = ml_dtypes.bfloat16
    in_maps = []
    for i in range(NC):
        b, g = i // 2, i % 2
        sl = slice(g * EH, (g + 1) * EH)
        in_maps.append({
            "xT": np.ascontiguousarray(x[b].T),
            "wqT": np.ascontiguousarray(Wq[sl, :].T),
            "wkT": np.ascontiguousarray(Wk[sl, :].T),
            "wvT": np.ascontiguousarray(Wv[sl, :].T) / 32.0,
            "woT": np.ascontiguousarray(Wo[:, sl].T).astype(bf),
            "bq": bq[sl].reshape(4, 128, 1).astype(np.float32),
            "bk": bk[sl].reshape(4, 128, 1).astype(np.float32),
        })
    return in_maps


def gather(results, Wv_b, Wo, bv, bo):
    host_bias = (bo + Wo @ bv / 32.0).astype(np.float32)
    out = np.empty((B, N, E), np.float32)
    for b in range(B):
        out[b] = (results[2 * b]["y"].astype(np.float32)
                  + results[2 * b + 1]["y"].astype(np.float32) + host_bias)
    return out


def kernel(x, Wq, bq, Wk, bk, Wv, bv, Wo, bo):
    x, Wq, bq, Wk, bk, Wv, bv, Wo, bo = [
        np.asarray(a, np.float32) for a in (x, Wq, bq, Wk, bk, Wv, bv, Wo, bo)]
    nc = build()
    in_maps = make_in_maps(x, Wq, bq, Wk, bk, Wv, bv, Wo, bo)
    res = run_bass_kernel_spmd(nc, in_maps, list(range(NC)))
    return gather(res.results, Wv, Wo, bv, bo)


if __name__ == "__main__":
    import reference
    inputs = {k: np.asarray(v) for k, v in reference.setup_inputs().items()}
    out = kernel(**inputs)
    exp = np.asarray(reference.reference(**inputs))
    rel = np.abs(out - exp).max() / np.abs(exp).max()
    print("Relative error:", rel)


# revision 4
# speedup vs baseline: 1.0945x; 1.0169x over previous
"""Distributed MHA kernel for 8 Trainium2 NeuronCores (v2).

Sharding: core i handles batch b = i//2, head-group g = i%2 (8 of 16 heads).
Data parallel on B, tensor parallel on H: column-parallel QKV, row-parallel
output projection with the partial sums reduced on the host during gather.

Math (per core, heads h in its group, E=1024, H=16, d=64, N=1024):
  QT[hd, n] = sum_e Wq[hd, e] x[n, e] + bq[hd]        (transposed layout)
  KT[hd, n] = likewise
  V[n, hd]  = sum_e x[n, e] Wv[hd, e] / 32             (bv folded on host,
                                                        1/32 att scale folded)
  energyT_h[k, q] = sum_d KT_h[d, k] QT_h[d, q]
  expT_h = exp(energyT_h)          (no max-subtract; |energy| < ~50 is safe)
  po_h[q, 0:64] = sum_k expT_h[k, q] V_h[k, d]         (full-K matmul,
  po_h[q, 64]   = sum_k expT_h[k, q]                    ones column in V)
  norm_h[q, d]  = po_h[q, 0:64] * (1/po_h[q, 64])      (per-partition scalar)
  packT[he, q]  = norm[q, he].T                        (DMA crossbar transpose)
  y_part[q, e]  = sum_{he} packT[he, q] Wo[e, he]
Host: out[b] = y_part[2b] + y_part[2b+1] + (bo + Wo @ bv / 32)

Schedule: the Act engine (exp, ~64us) paces stage B; QKV/V/output-projection
matmuls are interleaved into the energy stream so the PE never idles long.
Energy matmuls for the two heads of a pair sit on PE row-groups (0,0)/(64,0)
and run concurrently on hardware.
"""

import numpy as np
import ml_dtypes

import concourse.bass as bass
import concourse.tile as tile
from concourse import mybir
from concourse.bass_utils import run_bass_kernel_spmd

E = 1024
N = 1024
B = 4
NC = 8
EH = 512          # head dims per core (8 heads x 64)
D = 64
BF16 = mybir.dt.bfloat16
F32 = mybir.dt.float32
F32R = mybir.dt.float32r
AX = mybir.AluOpType
ACT = mybir.ActivationFunctionType


def split_drain_waits(nc):
    """Walrus in this toolchain rejects instructions carrying more than one
    sem wait; move extra waits onto injected same-engine NOPs placed right
    before the instruction (same engine queue = program order preserved)."""
    def take_nop(engine):
        nop = nc.engines[engine].nop(nofuse=True).ins
        for bname, bw in nc.bb_map.items():
            lst = bw.bb.instructions
            if lst and lst[-1].name == nop.name:
                bw.bb.instructions = lst[:-1]
                break
        return nop

    for name, w in nc.bb_map.items():
        bb = w.bb
        new_insts = []
        changed = False
        for ins in bb.instructions:
            si = ins.sync_info
            if si is not None and si.on_wait and len(si.on_wait) > 1:
                waits = list(si.on_wait)
                for wt in waits[:-1]:
                    nop = take_nop(ins.engine)
                    nop.sync_info = mybir.SyncInfo(on_wait=[wt], on_update=[])
                    new_insts.append(nop)
                si.on_wait = waits[-1:]
                ins.sync_info = si
                changed = True
            new_insts.append(ins)
        if changed:
            bb.instructions = new_insts


def _emit(nc: bass.Bass, tc: tile.TileContext, ctx):
    xT = nc.declare_dram_parameter("xT", [E, N], F32R, isOutput=False)
    wqT = nc.declare_dram_parameter("wqT", [E, EH], F32R, isOutput=False)
    wkT = nc.declare_dram_parameter("wkT", [E, EH], F32R, isOutput=False)
    wvT = nc.declare_dram_parameter("wvT", [E, EH], F32R, isOutput=False)
    woT = nc.declare_dram_parameter("woT", [EH, E], BF16, isOutput=False)
    bqd = nc.declare_dram_parameter("bq", [128, 4], F32, isOutput=False)
    bkd = nc.declare_dram_parameter("bk", [128, 4], F32, isOutput=False)
    y = nc.declare_dram_parameter("y", [N, E], BF16, isOutput=True)

    persist = ctx.enter_context(tc.tile_pool(name="persist", bufs=1))
    expool = ctx.enter_context(tc.tile_pool(name="expool", bufs=1))
    work = ctx.enter_context(tc.tile_pool(name="work", bufs=1))
    ps = ctx.enter_context(tc.tile_pool(name="ps", bufs=1, space="PSUM"))

    # ---- input DMA (3 queues: SP, Act-HWDGE, gpsimd SWDGE) ----
    # Per-DMA overheads are large (~565ns issue + 650ns DGE + 900ns sem), so
    # weights load as single rearranged-AP DMAs.  x loads in column halves so
    # the first energy tiles only wait on half of x.
    bq_t = persist.tile([128, 4], F32, tag="bq", name="bq_t")
    nc.gpsimd.dma_start(out=bq_t, in_=bqd[:, :])
    bk_t = persist.tile([128, 4], F32, tag="bk", name="bk_t")
    nc.gpsimd.dma_start(out=bk_t, in_=bkd[:, :])
    bq_sb = [bq_t[:, m:m + 1] for m in range(4)]
    bk_sb = [bk_t[:, m:m + 1] for m in range(4)]

    wq0_t = persist.tile([128, 8, 128], F32R, tag="wq0", name="wq0_t")
    nc.sync.dma_start(out=wq0_t,
                      in_=wqT[:, 0:128].rearrange("(c p) n -> p c n", p=128))
    wk0_t = persist.tile([128, 8, 128], F32R, tag="wk0", name="wk0_t")
    nc.scalar.dma_start(out=wk0_t,
                        in_=wkT[:, 0:128].rearrange("(c p) n -> p c n", p=128))
    xt = [persist.tile([128, N], F32R, tag=f"xt{e}", name=f"xt{e}")
          for e in range(8)]
    qmap = {0: nc.sync, 1: nc.scalar, 2: nc.sync, 3: nc.scalar,
            4: nc.sync, 5: nc.scalar, 6: nc.sync, 7: nc.scalar}
    for half in range(2):
        for e in range(8):
            qmap[e].dma_start(
                out=xt[e][:, half * 512:(half + 1) * 512],
                in_=xT[e * 128:(e + 1) * 128, half * 512:(half + 1) * 512])
    wqr_t = persist.tile([128, 8, 384], F32R, tag="wqr", name="wqr_t")
    nc.sync.dma_start(out=wqr_t,
                      in_=wqT[:, 128:512].rearrange("(c p) n -> p c n", p=128))
    wkr_t = persist.tile([128, 8, 384], F32R, tag="wkr", name="wkr_t")
    nc.scalar.dma_start(out=wkr_t,
                        in_=wkT[:, 128:512].rearrange("(c p) n -> p c n", p=128))
    wqm = [[wq0_t[:, e, :], wqr_t[:, e, 0:128], wqr_t[:, e, 128:256],
            wqr_t[:, e, 256:384]] for e in range(8)]
    wkm = [[wk0_t[:, e, :], wkr_t[:, e, 0:128], wkr_t[:, e, 128:256],
            wkr_t[:, e, 256:384]] for e in range(8)]
    wv_t = persist.tile([128, 8, EH], F32R, tag="wv", name="wv_t")
    nc.gpsimd.dma_start(out=wv_t,
                        in_=wvT[:, :].rearrange("(c p) n -> p c n", p=128))
    wv = [wv_t[:, e, :] for e in range(8)]
    wo_t = persist.tile([128, 4, E], BF16, tag="wo", name="wo_t")
    nc.sync.dma_start(out=wo_t,
                      in_=woT[:, :].rearrange("(c p) n -> p c n", p=128))
    wo = [wo_t[:, p, :] for p in range(4)]

    # Warm the Exp activation table during the DMA head.
    warm = persist.tile([1, 8], F32, tag="warm", name="warm")
    nc.vector.memset(warm, 0.0)
    nc.scalar.activation(out=warm, in_=warm, func=ACT.Exp)

    # persistent compute tiles
    qt = [persist.tile([128, N], F32R, tag=f"qt{m}", name=f"qt{m}") for m in range(4)]
    kt = [persist.tile([128, N], F32R, tag=f"kt{m}", name=f"kt{m}") for m in range(4)]
    vt = [persist.tile([128, 8, 65], BF16, tag=f"v{n}", name=f"v{n}") for n in range(8)]
    packT = [[persist.tile([128, 128], BF16, tag=f"pk{p}_{q}", name=f"pk{p}_{q}")
              for q in range(8)] for p in range(4)]

    # ---- PE work chunks (fillers interleaved into the act-paced B stream) --

    def emit_qk(m):
        """QK projections for pair m as 4 lazily-allocating PE chunks.

        PSUM tiles MUST be allocated inside the chunk bodies (at emission
        position) so tag-slot rotation order matches instruction order."""
        cell = {}

        def mm(dst, wsel, half):
            for e in range(8):
                w = wqm[e][m] if wsel == 0 else wkm[e][m]
                nc.tensor.matmul(
                    out=dst[:, half * 512:(half + 1) * 512],
                    lhsT=w, rhs=xt[e][:, half * 512:(half + 1) * 512],
                    start=(e == 0), stop=(e == 7))

        def bias(dst, src, b_sb):
            for half in range(2):
                nc.vector.tensor_scalar_add(
                    dst[:, half * 512:(half + 1) * 512],
                    src[:, half * 512:(half + 1) * 512], b_sb)

        def bias_h(dst, src, b_sb, half):
            nc.vector.tensor_scalar_add(
                dst[:, half * 512:(half + 1) * 512],
                src[:, half * 512:(half + 1) * 512], b_sb)

        # half-0 of Q and K complete first so the first four energy k-tiles
        # (keys 0:512) can start after only half of x has landed
        def c0():
            cell["psq"] = ps.tile([128, 1024], F32, tag="en", bufs=2,
                                  name=f"psq{m}")
            mm(cell["psq"], 0, 0)

        def c1():
            cell["psk"] = ps.tile([128, 1024], F32, tag="en", bufs=2,
                                  name=f"psk{m}")
            mm(cell["psk"], 1, 0)
            bias_h(qt[m], cell["psq"], bq_sb[m], 0)
            bias_h(kt[m], cell["psk"], bk_sb[m], 0)

        def c2():
            mm(cell["psq"], 0, 1)

        def c3():
            mm(cell["psk"], 1, 1)
            bias_h(qt[m], cell["psq"], bq_sb[m], 1)
            bias_h(kt[m], cell["psk"], bk_sb[m], 1)

        return [c0, c1, c2, c3]

    def emit_v(n):
        def go():
            psv = ps.tile([128, 512], F32, tag=f"cs{n % 2}", bufs=1, name=f"psv{n}")
            for e in range(8):
                nc.tensor.matmul(
                    out=psv, lhsT=xt[e][:, n * 128:(n + 1) * 128], rhs=wv[e],
                    start=(e == 0), stop=(e == 7))
            nc.vector.memset(vt[n][:, :, 64:65], 1.0)
            nc.vector.tensor_copy(
                vt[n][:, :, 0:64], psv.rearrange("p (h d) -> p h d", h=8))
        return go

    ys_ctr = [0]

    def emit_c(qt_i, es, copy_eng):
        def go():
            psy = ps.tile([128, 512], F32, tag=f"cs{es}", bufs=1, name=f"psy{qt_i}_{es}")
            for p in range(4):
                nc.tensor.matmul(
                    out=psy, lhsT=packT[p][qt_i],
                    rhs=wo[p][:, es * 512:(es + 1) * 512],
                    start=(p == 0), stop=(p == 3))
            ys = work.tile([128, 512], BF16, tag="ys", bufs=4,
                           name=f"ys{ys_ctr[0]}")
            ys_ctr[0] += 1
            if copy_eng == "act":
                nc.scalar.copy(ys, psy)
            else:
                nc.vector.tensor_copy(ys, psy)
            nc.sync.dma_start(
                out=y[qt_i * 128:(qt_i + 1) * 128, es * 512:(es + 1) * 512],
                in_=ys)
        return go

    # ---- stage B machinery ----
    def emit_en_exp(p, qs, k):
        """Energy for both heads of pair p (concurrent PE row groups) + exp."""
        en = ps.tile([128, 1024], F32, tag="en", bufs=2, name=f"en{p}_{qs}_{k}")
        for ab in range(2):
            nc.tensor.matmul(
                out=en[:, ab * 512:(ab + 1) * 512],
                lhsT=kt[p][ab * 64:(ab + 1) * 64, k * 128:(k + 1) * 128],
                rhs=qt[p][ab * 64:(ab + 1) * 64, qs * 512:(qs + 1) * 512],
                start=True, stop=True)
        ex = expool.tile([128, 1024], BF16, tag="ex", bufs=12,
                         name=f"ex{p}_{qs}_{k}")
        nc.scalar.activation(out=ex, in_=en, func=ACT.Exp)
        return ex

    def emit_attv(p, qs, qq, exs, po):
        """One qq-slice of att@V for both heads: full k-accumulation so each
        PSUM bank has a single pending accumulation group at a time."""
        for ab in range(2):
            for k in range(8):
                nc.tensor.matmul(
                    out=po[ab][:, qq, :],
                    lhsT=exs[k][:, ab * 512 + qq * 128:ab * 512 + (qq + 1) * 128],
                    rhs=vt[k][:, 2 * p + ab, :],
                    start=(k == 0), stop=(k == 7))

    def emit_norm_qq(p, qs, qq, po):
        """recip + per-partition scale for one qq, pack two heads, transpose."""
        s2 = work.tile([128, 2], F32, tag="s2", bufs=4, name=f"s2_{p}_{qs}_{qq}")
        nt = work.tile([128, 128], BF16, tag="norm", bufs=6,
                       name=f"nt{p}_{qs}_{qq}")
        for ab in range(2):
            nc.vector.reciprocal(out=s2[:, ab:ab + 1], in_=po[ab][:, qq, 64:65])
            nc.vector.tensor_scalar_mul(
                nt[:, ab * 64:(ab + 1) * 64],
                po[ab][:, qq, 0:64], s2[:, ab:ab + 1])
        nc.sync.dma_start_transpose(packT[p][qs * 4 + qq], nt)

    # ---- the act-paced pipeline ----
    # qk(p) due before group index p; V due gi0 (attV(0,0) runs during gi1).
    groups = [(p, qs) for qs in range(2) for p in range(4)]
    qk1 = emit_qk(1)
    vs = [emit_v(n) for n in range(8)]
    fillers = [qk1[0], vs[0], qk1[1], vs[1], qk1[2], vs[2], qk1[3], vs[3]]
    fillers += vs[4:]
    fillers += emit_qk(2) + emit_qk(3)
    # pops per (gi, k) from the fillers queue
    fill_sched = {
        0: [2, 2, 2, 2, 1, 1, 1, 1],          # qk1 + V (12 chunks)
        1: [1, 0, 1, 0, 1, 0, 1, 0],          # qk2 (due before gi2)
        2: [1, 0, 1, 0, 1, 0, 1, 0],          # qk3 (due before gi3)
    }
    # C chunks for qs0: legal once pair-3-qs0 transposes land (during gi4);
    # spread over gi5-gi7.  qs1 in the tail.
    c_qs0 = [emit_c(q, es, "dve") for q in range(4) for es in range(2)]
    c_qs1 = [emit_c(q, es, "act") for q in range(4, 8) for es in range(2)]
    c_sched = {
        5: [0, 1, 0, 1, 0, 1, 0, 1],
        6: [0, 1, 0, 1, 0, 1, 0, 1],
    }

    for ch in emit_qk(0):
        ch()

    prev = None          # (p, qs, ex_tiles, po)
    fill_i = 0
    for gi, (p, qs) in enumerate(groups):
        po = [ps.tile([128, 4, 65], F32, tag=f"po{ab}", bufs=1,
                      name=f"po{p}_{qs}_{ab}") for ab in range(2)]
        exs = []
        for k in range(8):
            ex = emit_en_exp(p, qs, k)
            exs.append(ex)
            if prev is not None and k % 2 == 1:
                pp, pqs, pexs, ppo = prev
                qq = (k - 1) // 2
                emit_attv(pp, pqs, qq, pexs, ppo)
                if k == 7:
                    for q4 in range(4):
                        emit_norm_qq(pp, pqs, q4, ppo)
            for _ in range(fill_sched.get(gi, [0] * 8)[k]):
                if fill_i < len(fillers):
                    fillers[fill_i]()
                    fill_i += 1
            for _ in range(c_sched.get(gi, [0] * 8)[k]):
                if c_qs0:
                    c_qs0.pop(0)()
        prev = (p, qs, exs, po)

    # tail: attV + norm of the last group interleaved with the final C chunks
    pp, pqs, pexs, ppo = prev
    for qq in range(4):
        emit_attv(pp, pqs, qq, pexs, ppo)
        emit_norm_qq(pp, pqs, qq, ppo)
        if qq >= 1:
            c_qs1.pop(0)()          # C(qt4+qq-1, es0) once its packT landed
            c_qs1.pop(0)()
    while fill_i < len(fillers):
        fillers[fill_i]()
        fill_i += 1
    for ch in c_qs0:
        ch()
    for ch in c_qs1:
        ch()


def build(split=True):
    from contextlib import ExitStack
    nc = bass.Bass()
    with tile.TileContext(nc) as tc:
        with ExitStack() as ctx:
            _emit(nc, tc, ctx)
    if split:
        split_drain_waits(nc)
    return nc


def make_in_maps(x, Wq, bq, Wk, bk, Wv, bv, Wo, bo):
    bf = ml_dtypes.bfloat16
    in_maps = []
    for i in range(NC):
        b, g = i // 2, i % 2
        sl = slice(g * EH, (g + 1) * EH)
        in_maps.append({
            "xT": np.ascontiguousarray(x[b].T),
            "wqT": np.ascontiguousarray(Wq[sl, :].T),
            "wkT": np.ascontiguousarray(Wk[sl, :].T),
            "wvT": np.ascontiguousarray(Wv[sl, :].T) / 32.0,
            "woT": np.ascontiguousarray(Wo[:, sl].T).astype(bf),
            "bq": np.ascontiguousarray(bq[sl].reshape(4, 128).T).astype(np.float32),
            "bk": np.ascontiguousarray(bk[sl].reshape(4, 128).T).astype(np.float32),
        })
    return in_maps


def gather(results, Wv_b, Wo, bv, bo):
    host_bias = (bo + Wo @ bv / 32.0).astype(np.float32)
    out = np.empty((B, N, E), np.float32)
    for b in range(B):
        out[b] = (results[2 * b]["y"].astype(np.float32)
                  + results[2 * b + 1]["y"].astype(np.float32) + host_bias)
    return out


def kernel(x, Wq, bq, Wk, bk, Wv, bv, Wo, bo):
    x, Wq, bq, Wk, bk, Wv, bv, Wo, bo = [
        np.asarray(a, np.float32) for a in (x, Wq, bq, Wk, bk, Wv, bv, Wo, bo)]
    nc = build()
    in_maps = make_in_maps(x, Wq, bq, Wk, bk, Wv, bv, Wo, bo)
    res = run_bass_kernel_spmd(nc, in_maps, list(range(NC)))
    return gather(res.results, Wv, Wo, bv, bo)


if __name__ == "__main__":
    import reference
    inputs = {k: np.asarray(v) for k, v in reference.setup_inputs().items()}
    out = kernel(**inputs)
    exp = np.asarray(reference.reference(**inputs))
    rel = np.abs(out - exp).max() / np.abs(exp).max()
    print("Relative error:", rel)
